# revision 41
# baseline (speedup 1.0000x reference)
"""Causal self-attention (with the reference's inverted mask) on 8 TRN2
NeuronCores.

Problem (hardcoded): B=2, S=2048, D=1024, H=16 heads, head_dim=64, fp32.
  q/k/v = x @ W* + b*;  score = q k^T / 8;  score += tril(ones)*(-1e9)
  (inverted causal mask: the LOWER triangle incl. diagonal is masked, so
  softmax attends strictly to k > q; row q=S-1 is fully masked and its
  softmax is exactly uniform, since all its masked inputs round to exactly
  -1e9 in fp32);  out = softmax(score) @ v @ Wo + bo.

Sharding: core c handles batch b = c//4 and heads [4*(c%4), 4*(c%4)+4).
Each core computes a partial output (its 4 heads' slice of attn @ Wo);
the host sums 4 partials per batch and adds bo.

Per-core kernel (all matmul operands bf16, fp32 PSUM accumulation:
~4.6e-3 rel err vs the fp32 reference, well under the 2e-2 gate; bf16
registers HAM activity so the PE sustains 2.4 GHz instead of the 1.2
GHz mid p-state that f32r was stuck at — this halved the kernel time):
  Phase A: QT/KT = W^T x^T in [dh, s] layout (head pairs packed to 128
    partitions), V in [s, dh] layout with an extra ones column per head
    ([V | 1]) so one matmul later yields both the attn numerator and the
    softmax denominator.  Projections run n-major (per 512-col x
    quarter) so the PE starts as soon as quarter 0 lands; x is loaded
    quarter-major with host-prepacked [n,p,c,s] layout (8KB contiguous
    DMA descriptors); non-critical loads are gated on 1-element marker
    writes so the DGE can't steal bandwidth from the critical prefix.
    START_DUMMIES bf16 matmuls burn the HAM ramp during the DMA fill.
    Chunk0's k-blocks j=0..11 and chunk1's diagonal j=4..7 are ALSO
    scored+exp'd here, interleaved between projection groups into the
    persistent ptbuf: without this, chunk0 is ACT-bound (41us of exp vs
    29us of PE), since exp on ACT costs ~2.2us per off-diag k-block vs
    1.7us of PE work.
  Phase B (per q-chunk of 512): scores computed TRANSPOSED,
    s^T[k, q] = K^T Q per (head, k-block j), so softmax needs no
    max-subtraction and no transposes of the probability matrix:
    p^T = exp(s^T/8) in bf16 (masked entries zero-filled via a bf16 DVE
    mask multiply, matching the reference where exp(-1e9 - max)
    underflows to exactly 0).  Only k-blocks j >= 4c are active, and
    diagonal blocks narrow to 128(d+1) q-columns.
    attn^T[dh|sum, q] accumulates matmul([V|1], p^T) over j in PSUM.
    The globally-masked last row (q=2047) is exactly uniform attention;
    recomputed exactly on the host (kept finite on-chip via [0|1]
    matmuls into columns 510:512).
    Normalization: broadcast the sums row to 64 partitions with a K=1
    ones matmul (bcs tiles live in the psO pool so the next chunk's
    scores never wait on a late reciprocal read of a psS bank), 64-lane
    reciprocal_approx_fast, multiply.  Odd heads go through a
    partition-shifting SBUF DMA into rows 64:128 of the pair tile.
  Phase C (fused per q-chunk): out_partial[s-blocks, :] = attn^T.T @
    Wo-rows, K=128 pair contraction; the first psum groups of each
    chunk are pre-opened with zero-add dummy matmuls (0-valued
    stationary) that fill the PE bubble while the norm chain drains and
    keep HAM from re-throttling; both 512-col halves land in one ob
    tile and ship as a single contiguous [128, 1024] DMA per s-block.
"""

import numpy as np

B, S, D, H, DH = 2, 2048, 1024, 16, 64
HPC = 4                 # heads per core
NCORES = 8
NPAIR = HPC // 2        # head pairs per core (2)
SBLK = S // 128         # 16 s/k blocks
NCH = S // 512          # 4 q-chunks of 512
CHUNKS = D // 128       # 8 contraction chunks of the model dim
START_DUMMIES = 20      # bf16 HAM warmers during the initial DMA fill
CWARM = 4               # zero-add matmuls pre-opening each C psum group

_CACHE = {}


def _build_nc(debug=False):
    import concourse.mybir as mybir
    from concourse import bacc, tile

    f32 = mybir.dt.float32
    bf16 = mybir.dt.bfloat16
    AF = mybir.ActivationFunctionType
    OP = mybir.AluOpType

    nc = bacc.Bacc("TRN2", target_bir_lowering=False)

    # x^T pre-quartered host-side: xq[n, p, c, s] = x^T[128c+p, 512n+s].
    # [n, p, c, s] order makes each quarter's DMA fully contiguous per
    # partition (8 KB descriptors instead of 1 KB).
    xq = nc.dram_tensor("xq", [NCH, 128, CHUNKS, 512], bf16,
                        kind="ExternalInput")
    # weights pre-packed host-side to the on-chip tile layouts
    wq = nc.dram_tensor("wq", [128, CHUNKS, HPC * DH], bf16,
                        kind="ExternalInput")
    wk = nc.dram_tensor("wk", [128, CHUNKS, HPC * DH], bf16,
                        kind="ExternalInput")
    wv = nc.dram_tensor("wv", [128, CHUNKS, HPC * DH], bf16,
                        kind="ExternalInput")
    wo = nc.dram_tensor("wo", [128, NPAIR, D], bf16, kind="ExternalInput")
    # per-pair q/k biases: [128, 4] cols = (q pair0, q pair1, k pair0, k pair1)
    bqk = nc.dram_tensor("bqk", [128, 2 * NPAIR], f32, kind="ExternalInput")
    # bv broadcast to all partitions host-side: [128, 256]
    bvb = nc.dram_tensor("bvb", [128, HPC * DH], f32, kind="ExternalInput")
    # diagonal-block causal masks: masks[k_local, d, f] = (128d + k_local > f)
    masks = nc.dram_tensor("masks", [128, 4, 512], bf16,
                           kind="ExternalInput")
    out = nc.dram_tensor("out", [S, D], f32, kind="ExternalOutput")
    if debug:
        qt_d = nc.dram_tensor("qt_d", [128, NPAIR, S], f32,
                              kind="ExternalOutput")
        kt_d = nc.dram_tensor("kt_d", [128, NPAIR, S], f32,
                              kind="ExternalOutput")
        vsb_d = nc.dram_tensor("vsb_d", [128, SBLK, HPC, DH + 1], f32,
                               kind="ExternalOutput")
        atn_d = nc.dram_tensor("atn_d", [128, NCH, NPAIR, 512], f32,
                               kind="ExternalOutput")
        psa_d = nc.dram_tensor("psa_d", [DH + 1, NCH, HPC, 512], f32,
                               kind="ExternalOutput")

    with tile.TileContext(nc) as tc:
        with (
            tc.tile_pool(name="pers", bufs=1) as pers,
        ):
            misc = pers
            # Q^T head pairs, two variants with the other head's rows
            # zeroed so score matmuls can contract K=128 (K=64 matmuls
            # register HAM activity poorly and can run at 1.2 GHz).
            # One tile [128, variant, pair, S] so diagonal blocks can run
            # both halves in a single matmul with a strided moving AP.
            qz2 = pers.tile([128, 2, NPAIR, S], bf16)
            kt = pers.tile([128, NPAIR, S], bf16)         # K^T head pairs
            vsb = pers.tile([128, SBLK, HPC, DH + 1], bf16)  # [V | 1]
            wo_t = pers.tile([128, NPAIR, D], bf16)       # Wo head pairs
            ones2 = misc.tile([128, 2], bf16)   # [0 | 1] columns
            onef = misc.tile([128, 2], f32)
            onesrow = misc.tile([DH + 1, DH], bf16)  # row 64 = ones
            bias_t = misc.tile([128, 2 * NPAIR], f32)
            bvb_t = misc.tile([128, HPC * DH], f32)
            mask_t = misc.tile([128, 4, 512], bf16)

            dmy_w = misc.tile([128, 128], bf16)
            dmy_z = misc.tile([128, 128], bf16)   # zero stationary: +0 adds
            dmy_m = misc.tile([128, 512], bf16)
            nc.gpsimd.memset(dmy_w[:], 0.25)
            nc.gpsimd.memset(dmy_z[:], 0.0)
            nc.gpsimd.memset(dmy_m[:], 0.25)
            nc.sync.dma_start(bias_t[:], bqk[:])
            nc.gpsimd.memset(onef[:, 0:1], 0.0)
            nc.gpsimd.memset(onef[:, 1:2], 1.0)
            nc.vector.tensor_copy(ones2[:], onef[:])  # bf16 [0|1]
            nc.vector.tensor_copy(
                onesrow[DH:DH + 1, :],
                onef[DH:DH + 1, 1:2].to_broadcast((1, DH)))
            # ones column of [V|1] for every (sblk, head)
            nc.vector.tensor_copy(
                vsb[:, :, :, DH:DH + 1],
                onef[:, 1:2].to_broadcast((128, SBLK, HPC, 1)))
            nc.vector.tensor_copy(
                qz2[64:128, 0, :, :],
                onef[64:128, 0:1].to_broadcast((64, NPAIR, S)))
            nc.vector.tensor_copy(
                qz2[0:64, 1, :, :],
                onef[0:64, 0:1].to_broadcast((64, NPAIR, S)))

            # Pre-computed exp(score) tiles: chunk0's j=0..11 and chunk1's
            # diagonal j=4..7 are scored+exp'd DURING phase A (interleaved
            # between projection groups) so the ACT engine's exp backlog
            # starts draining while the PE is projection-bound.  Without
            # this, chunk0 is ACT-bound (41us of exp vs 29us of matmul).
            PRE = ([(0, j, j) for j in range(12)]
                   + [(1, j, 8 + j) for j in range(4, 8)]
                   + [(3, j, 4 + j) for j in range(12, 16)])
            # slot-reuse prescores emitted DURING phase B (slots freed by
            # chunk0's early attnV reads): ch2's diag into slots 0..3,
            # ch3's diag into slots 8..11.  Emission points (ch, idx)
            # are placed AFTER the attnV that frees the slot, so the
            # ACT exp never inverts the PE FIFO order (deadlock-safe).
            PRE_B = {}
            pre_map = {(c_, j_): s_ for c_, j_, s_ in PRE}
            for blks in PRE_B.values():
                for c_, j_, s_ in blks:
                    pre_map[(c_, j_)] = s_
            ptbuf = pers.tile([128, 20, HPC, 512], bf16)

            with (
                tc.tile_pool(name="psS", bufs=2, space="PSUM") as psS,
            ):
                def emit_score_block(ch, j, pts4):
                    """scores + exp + causal mask for one (chunk, k-block):
                    4 head tiles of exp(score^T/8) land in pts4."""
                    d = j - 4 * ch
                    W = 128 * (d + 1) if d < 4 else 512
                    for pair in range(NPAIR):
                        merged = d < 2  # both halves in one matmul
                        if merged:
                            pss = psS.tile([128, 512], f32, tag="pss",
                                           name="pss")
                            nc.tensor.matmul(
                                pss[:].rearrange(
                                    "p (v w) -> p v w", v=2)[:, :, 0:W],
                                kt[:, pair, 128 * j:128 * j + 128],
                                qz2[:, :, pair, 512 * ch:512 * ch + W],
                                start=True, stop=True)
                        for half in range(2):
                            h = 2 * pair + half
                            if not merged:
                                pss = psS.tile([128, 512], f32,
                                               tag="pss", name="pss")
                                nc.tensor.matmul(
                                    pss[:, 0:W],
                                    kt[:, pair, 128 * j:128 * j + 128],
                                    qz2[:, half, pair,
                                        512 * ch:512 * ch + W],
                                    start=True, stop=True)
                                src_ap = pss[:, 0:W]
                            else:
                                src_ap = pss[:, 256 * half:256 * half + W]
                            nc.scalar.activation(pts4[h][:, 0:W], src_ap,
                                                 AF.Exp, scale=0.125)
                            if d < 4:
                                # zero where k <= q (DVE mask multiply)
                                nc.vector.tensor_tensor(
                                    pts4[h][:, 0:W], pts4[h][:, 0:W],
                                    mask_t[:, d, 0:W], op=OP.mult)

                # ---------------- Phase A: projections ----------------
                ctxA = nc.named_scope("phaseA"); ctxA.__enter__()
                with (
                    tc.tile_pool(name="wts", bufs=1) as wts,
                    tc.tile_pool(name="psA", bufs=2,
                                 space="PSUM") as psA,
                ):
                    psV = psA
                    psW = psA
                    dmy_ps = psW.tile([128, 512], f32, name="dmy_ps",
                                      tag="dmy", bufs=1)
                    # quarter-major x so each quarter's DMA is contiguous
                    xtr = wts.tile([128, NCH, CHUNKS, 512], bf16)
                    wq_t = wts.tile([128, CHUNKS, HPC * DH], bf16,
                                    tag="wq")
                    wk_t = wts.tile([128, CHUNKS, HPC * DH], bf16,
                                    tag="wk")
                    wv_t = wts.tile([128, CHUNKS, HPC * DH], bf16,
                                    tag="wv")

                    # DMA priority: the DGE round-robins descriptors of
                    # every queued transfer, so simply emitting the
                    # critical loads first is NOT enough — later loads
                    # steal bandwidth.  Gate each non-critical load on a
                    # 1-element marker write into its destination whose
                    # source is a projection output: the DMA then can't
                    # start until the pipeline actually needs it.
                    nc.sync.dma_start(wq_t[:], wq[:])
                    nc.sync.dma_start(mask_t[:], masks[:])
                    nc.sync.dma_start(bvb_t[:], bvb[:])
                    # x quarter 0 in chunk-pair sub-DMAs so the first
                    # projection group can start on partial data
                    for cp in range(4):
                        nc.scalar.dma_start(xtr[:, 0, 2 * cp:2 * cp + 2],
                                            xq[0, :, 2 * cp:2 * cp + 2])

                    def gated(engine, marker_dst, marker_src, dst_ap,
                              src_ap):
                        nc.vector.tensor_copy(marker_dst, marker_src)
                        engine.dma_start(dst_ap, src_ap)

                    # wk/wv unlock once the first Q evac lands (~3us in)
                    gated(nc.sync, wk_t[0:1, 0:1, 0:1],
                          qz2[0:1, 0, 0, 0:1], wk_t[:], wk[:])
                    gated(nc.scalar, wv_t[0:1, 0:1, 0:1],
                          qz2[0:1, 0, 0, 0:1], wv_t[:], wv[:])
                    # x quarter n unlocks on Q(p0, n-1)'s evac
                    gated(nc.sync, xtr[0:1, 1, 0:1, 0:1],
                          qz2[0:1, 0, 0, 0:1], xtr[:, 1], xq[1])
                    gated(nc.scalar, xtr[0:1, 2, 0:1, 0:1],
                          qz2[0:1, 0, 0, 512:513], xtr[:, 2], xq[2])
                    gated(nc.sync, xtr[0:1, 3, 0:1, 0:1],
                          qz2[0:1, 0, 0, 1024:1025], xtr[:, 3], xq[3])
                    # wo is first needed by chunk0's phase C, much later
                    gated(nc.scalar, wo_t[0:1, 0:1, 0:1],
                          qz2[0:1, 0, 0, 1536:1537], wo_t[:], wo[:])
                    # Burn the HAM ramp while the first loads land: ~3.4us
                    # of dummy bf16 matmuls brings the PE to 2.4 GHz right
                    # as the first projection group's inputs arrive.
                    for _ in range(START_DUMMIES):
                        nc.tensor.matmul(dmy_ps[:], dmy_w[:], dmy_m[:],
                                         start=True, stop=True)

                    def proj_group(dsts, p, n):
                        w_tile = wq_t if dsts == "q" else wk_t
                        bcol0 = 0 if dsts == "q" else NPAIR
                        ps = psA.tile([128, 512], f32, name="ps", tag="ps")
                        for c in range(CHUNKS):
                            nc.tensor.matmul(
                                ps[:],
                                w_tile[:, c, 128 * p:128 * p + 128],
                                xtr[:, n, c, :],
                                start=(c == 0), stop=(c == CHUNKS - 1))
                        # evacuate + add per-partition bias (dh rows)
                        sl = slice(512 * n, 512 * n + 512)
                        bias = bias_t[:, bcol0 + p:bcol0 + p + 1]
                        if dsts == "k":
                            nc.scalar.activation(
                                kt[:, p, sl], ps[:], AF.Identity,
                                bias=bias)
                        else:
                            nc.scalar.activation(
                                qz2[0:64, 0, p, sl], ps[0:64, :],
                                AF.Identity, bias=bias[0:64, :])
                            nc.scalar.activation(
                                qz2[64:128, 1, p, sl], ps[64:128, :],
                                AF.Identity, bias=bias[64:128, :])

                    # n-major so each quarter's Q/K/V needs only x quarter
                    # n; the PE starts on quarter 0 while 1-3 stream in.
                    # Pre-score blocks are interleaved between projection
                    # groups of the NEXT quarter (their K/Q deps are done).
                    pre_by_n = {1: PRE[0:4], 2: PRE[4:8],
                                3: PRE[8:16] + PRE[16:20]}
                    for n in range(NCH):
                        pre_list = list(pre_by_n.get(n, []))
                        pre_pts = [8, 8, 8, 8][n]

                        def pre_step(k=1):
                            for _ in range(k):
                                if pre_list:
                                    c_, j_, s_ = pre_list.pop(0)
                                    emit_score_block(
                                        c_, j_,
                                        [ptbuf[:, s_, h_, :]
                                         for h_ in range(HPC)])

                        for p in range(NPAIR):
                            proj_group("q", p, n)
                            pre_step()
                        for p in range(NPAIR):
                            proj_group("k", p, n)
                            pre_step()
                        for sb in range(4 * n, 4 * n + 4):
                            k = sb - 4 * n
                            ps = psV.tile([128, HPC * DH], f32,
                                          name="psv", tag="psv",
                                          bufs=2)
                            for c in range(CHUNKS):
                                nc.tensor.matmul(
                                    ps[:],
                                    xtr[:, n, c, 128 * k:128 * k + 128],
                                    wv_t[:, c, :],
                                    start=(c == 0),
                                    stop=(c == CHUNKS - 1))
                            nc.vector.tensor_tensor(
                                vsb[:, sb, :, 0:DH],
                                ps[:].rearrange("p (h d) -> p h d", h=HPC),
                                bvb_t[:].rearrange("p (h d) -> p h d",
                                                   h=HPC),
                                op=OP.add)
                            pre_step(2 if n == 3 else 1)

                ctxA.__exit__(None, None, None)
                # ------------- Phase B + fused C, per q-chunk -------------
                with (
                    tc.tile_pool(name="wrk", bufs=2) as wrk,
                    tc.tile_pool(name="psB", bufs=1,
                                 space="PSUM") as psB,
                ):
                    srowp = rcpp = toddp = obp = wrk
                    psAt = psO = psB
                    for ch in range(NCH):
                        ctxB = nc.named_scope(f"chunk{ch}")
                        ctxB.__enter__()
                        js = list(range(4 * ch, SBLK))
                        psa = [psAt.tile([DH + 1, 512], f32,
                                         tag=f"psa{h}", name=f"psa{h}")
                               for h in range(HPC)]
                        for idx, j in enumerate(js):
                            d = j - 4 * ch
                            W = 128 * (d + 1) if d < 4 else 512
                            slot = pre_map.get((ch, j))
                            if slot is not None:
                                pts = [ptbuf[:, slot, h, :]
                                       for h in range(HPC)]
                            else:
                                pts = [pers.tile([128, 512], bf16,
                                                 name="pt", tag="pt",
                                                 bufs=6)
                                       for _ in range(HPC)]
                                emit_score_block(ch, j, pts)
                            last = (idx == len(js) - 1) and ch < 3
                            for h in range(HPC):
                                nc.tensor.matmul(
                                    psa[h][:, 0:W], vsb[:, j, h, :],
                                    pts[h][:, 0:W],
                                    start=(idx == 0), stop=last,
                                    skip_group_check=(ch == 3))
                            for c_, j_, s_ in PRE_B.get((ch, idx), []):
                                emit_score_block(
                                    c_, j_,
                                    [ptbuf[:, s_, h_, :]
                                     for h_ in range(HPC)])
                        if ch == 3:
                            # last global row q=2047 is fully masked; its
                            # exact value (uniform attention = mean(V)@Wo)
                            # is recomputed on the host.  Keep column
                            # 511's denominator finite (one [0|1]-column
                            # matmul) to avoid Inf/NaN noise.
                            for h in range(HPC):
                                nc.tensor.matmul(
                                    psa[h][:, 510:512],
                                    vsb[:, 0, h, :], ones2[:],
                                    start=False, stop=True)
                        # normalize: attn^T rows / sums row.  Broadcast
                        # the sums row via a K=1 ones matmul, 64-lane
                        # approx reciprocal, then multiply.  Odd heads go
                        # through a SBUF tile and a partition-shifting DMA
                        # into rows 64:128 of the pair tile so phase C
                        # contracts K=128.  bcs tiles allocate from the
                        # psO pool (NOT psS) so the next chunk's first
                        # score matmul never waits on a late reciprocal
                        # read of a psS bank.
                        atn = pers.tile([128, NPAIR, 512], bf16,
                                        name="atn", tag="atn", bufs=2)
                        srows = []
                        for h in range(HPC):
                            srow = srowp.tile([DH + 1, 512], bf16)
                            nc.scalar.copy(srow[DH:DH + 1, :],
                                           psa[h][DH:DH + 1, :])
                            srows.append(srow)
                        rcps = []
                        for h in range(HPC):
                            bcs = psO.tile([128, 512], f32, name="bcs",
                                           tag="po", bufs=2)
                            # zero-add filler first: depends only on the
                            # psum bank, so the PE stays busy (and HAM
                            # stays hot) while the srow copy lands
                            nc.tensor.matmul(bcs[0:64, :],
                                             dmy_z[:, 0:64], dmy_m[:],
                                             start=True, stop=False)
                            nc.tensor.matmul(bcs[0:64, :],
                                             onesrow[DH:DH + 1, :],
                                             srows[h][DH:DH + 1, :],
                                             start=False, stop=True)
                            rcp = rcpp.tile([64, 512], f32)
                            nc.vector.reciprocal_approx_fast(rcp[:],
                                                             bcs[0:64, :])
                            rcps.append(rcp)
                        for h in range(HPC):
                            pair, half = h // 2, h % 2
                            if half == 0:
                                nc.vector.tensor_tensor(
                                    atn[0:64, pair, :], psa[h][0:DH, :],
                                    rcps[h][:], op=OP.mult)
                            else:
                                todd = toddp.tile([64, 512], bf16)
                                nc.vector.tensor_tensor(
                                    todd[:], psa[h][0:DH, :], rcps[h][:],
                                    op=OP.mult)
                                nc.sync.dma_start(atn[64:128, pair, :],
                                                  todd[:])

                        # fused phase C for this chunk's 4 s-blocks.  The
                        # first two psum groups are pre-opened with
                        # zero-add dummy matmuls: they depend only on free
                        # psO banks, so they fill the PE bubble while the
                        # norm chain (copy -> bcast -> rcp -> mult ->
                        # shift DMA) drains, and keep HAM warm.
                        for k in range(4):
                            sb = 4 * ch + k
                            ob = obp.tile([128, 2, 512], f32, name="ob",
                                          tag="ob", bufs=2)
                            for n in range(2):
                                ps = psO.tile([128, 512], f32,
                                              name="po", tag="po",
                                              bufs=2)
                                warm = (CWARM if k == 0 else
                                        (2 if ch == 3 else 0))
                                for w in range(warm):
                                    nc.tensor.matmul(
                                        ps[:], dmy_z[:], dmy_m[:],
                                        start=(w == 0), stop=False)
                                for p in range(NPAIR):
                                    nc.tensor.matmul(
                                        ps[:],
                                        atn[:, p, 128 * k:128 * k + 128],
                                        wo_t[:, p, 512 * n:512 * n + 512],
                                        start=(warm == 0 and p == 0),
                                        stop=(p == NPAIR - 1))
                                if n == 0:
                                    nc.scalar.copy(ob[:, 0, :], ps[:])
                                else:
                                    nc.vector.tensor_copy(ob[:, 1, :],
                                                          ps[:])
                            if ch == 3 and k == 3:
                                # final store: split across both rings so
                                # the drain isn\'t one serial 512KB DMA
                                nc.sync.dma_start(
                                    out[128 * sb:128 * sb + 128, 0:512],
                                    ob[:, 0, :])
                                nc.scalar.dma_start(
                                    out[128 * sb:128 * sb + 128,
                                        512:1024], ob[:, 1, :])
                            else:
                                nc.sync.dma_start(
                                    out[128 * sb:128 * sb + 128, :],
                                    ob[:].rearrange("p a b -> p (a b)"))
                        ctxB.__exit__(None, None, None)

    nc.finalize()
    return nc


def _prep_in_maps(inputs, Wq, bq, Wk, bk, Wv, bv, Wo, bo):
    import ml_dtypes
    bf = ml_dtypes.bfloat16
    in_maps = []
    # xq[n, p, c, s] = x^T[128c+p, 512n+s]: each (quarter, partition) is
    # a contiguous 8KB run, so the DMA gets full-size descriptors
    xqs = []
    for b in range(B):
        xT = np.ascontiguousarray(inputs[b].T).astype(bf)
        xqs.append(np.ascontiguousarray(
            xT.reshape(CHUNKS, 128, NCH, 512).transpose(2, 1, 0, 3)))
    kk = np.arange(128)[:, None, None]
    dd = np.arange(4)[None, :, None]
    ff = np.arange(512)[None, None, :]
    masks = ((128 * dd + kk) > ff).astype(bf)
    for core in range(NCORES):
        b = core // (NCORES // B)
        g = core % (NCORES // B)
        cols = slice(g * HPC * DH, (g + 1) * HPC * DH)
        bq_c = bq[cols].reshape(NPAIR, 128).T          # [128, 2]
        bk_c = bk[cols].reshape(NPAIR, 128).T
        bqk_c = np.ascontiguousarray(
            np.concatenate([bq_c, bk_c], axis=1), dtype=np.float32)
        bvb_c = np.ascontiguousarray(
            np.broadcast_to(bv[cols][None, :], (128, HPC * DH)),
            dtype=np.float32)

        def pack_w(w):  # [D, HPC*DH] -> [128, CHUNKS, HPC*DH]
            return np.ascontiguousarray(
                w.astype(bf).reshape(CHUNKS, 128, HPC * DH)
                .transpose(1, 0, 2))

        wo_c = np.ascontiguousarray(
            Wo[cols, :].astype(bf).reshape(NPAIR, 128, D)
            .transpose(1, 0, 2))
        in_maps.append({
            "xq": xqs[b],
            "wq": pack_w(np.ascontiguousarray(Wq[:, cols])),
            "wk": pack_w(np.ascontiguousarray(Wk[:, cols])),
            "wv": pack_w(np.ascontiguousarray(Wv[:, cols])),
            "wo": wo_c,
            "bqk": bqk_c,
            "bvb": bvb_c,
            "masks": masks,
        })
    return in_maps


def kernel(inputs, Wq, bq, Wk, bk, Wv, bv, Wo, bo, _want_results=False,
           **_run_kwargs):
    from concourse.bass_utils import run_bass_kernel_spmd

    inputs = np.asarray(inputs, dtype=np.float32)
    Wq, bq = np.asarray(Wq, np.float32), np.asarray(bq, np.float32)
    Wk, bk = np.asarray(Wk, np.float32), np.asarray(bk, np.float32)
    Wv, bv = np.asarray(Wv, np.float32), np.asarray(bv, np.float32)
    Wo, bo = np.asarray(Wo, np.float32), np.asarray(bo, np.float32)

    if "nc" not in _CACHE:
        _CACHE["nc"] = _build_nc()
    nc = _CACHE["nc"]

    in_maps = _prep_in_maps(inputs, Wq, bq, Wk, bk, Wv, bv, Wo, bo)
    res = run_bass_kernel_spmd(nc, in_maps, core_ids=list(range(NCORES)),
                               **_run_kwargs)

    out = np.zeros((B, S, D), dtype=np.float32)
    for core in range(NCORES):
        b = core // (NCORES // B)
        out[b] += res.results[core]["out"]
    out += bo[None, None, :]
    # exact last row (fully masked -> uniform attention = mean(V) @ Wo)
    for b in range(B):
        v_mean = inputs[b].mean(axis=0) @ Wv + bv
        out[b, S - 1, :] = v_mean @ Wo + bo
    if _want_results:
        return out, res
    return out



# revision 42
# speedup vs baseline: 1.0184x; 1.0184x over previous
"""Causal self-attention (with the reference's inverted mask) on 8 TRN2
NeuronCores.

Problem (hardcoded): B=2, S=2048, D=1024, H=16 heads, head_dim=64, fp32.
  q/k/v = x @ W* + b*;  score = q k^T / 8;  score += tril(ones)*(-1e9)
  (inverted causal mask: the LOWER triangle incl. diagonal is masked, so
  softmax attends strictly to k > q; row q=S-1 is fully masked and its
  softmax is exactly uniform, since all its masked inputs round to exactly
  -1e9 in fp32);  out = softmax(score) @ v @ Wo + bo.

Sharding: core c handles batch b = c//4 and heads [4*(c%4), 4*(c%4)+4).
Each core computes a partial output (its 4 heads' slice of attn @ Wo);
the host sums 4 partials per batch and adds bo.

Per-core kernel (all matmul operands bf16, fp32 PSUM accumulation:
~4.6e-3 rel err vs the fp32 reference, well under the 2e-2 gate; bf16
registers HAM activity so the PE sustains 2.4 GHz instead of the 1.2
GHz mid p-state that f32r was stuck at — this halved the kernel time):
  Phase A: QT/KT = W^T x^T in [dh, s] layout (head pairs packed to 128
    partitions), V in [s, dh] layout with an extra ones column per head
    ([V | 1]) so one matmul later yields both the attn numerator and the
    softmax denominator.  Projections run n-major (per 512-col x
    quarter) so the PE starts as soon as quarter 0 lands; x is loaded
    quarter-major with host-prepacked [n,p,c,s] layout (8KB contiguous
    DMA descriptors); non-critical loads are gated on 1-element marker
    writes so the DGE can't steal bandwidth from the critical prefix.
    START_DUMMIES bf16 matmuls burn the HAM ramp during the DMA fill.
    Chunk0's k-blocks j=0..11 and chunk1's diagonal j=4..7 are ALSO
    scored+exp'd here, interleaved between projection groups into the
    persistent ptbuf: without this, chunk0 is ACT-bound (41us of exp vs
    29us of PE), since exp on ACT costs ~2.2us per off-diag k-block vs
    1.7us of PE work.
  Phase B (per q-chunk of 512): scores computed TRANSPOSED,
    s^T[k, q] = K^T Q per (head, k-block j), so softmax needs no
    max-subtraction and no transposes of the probability matrix:
    p^T = exp(s^T/8) in bf16 (masked entries zero-filled via a bf16 DVE
    mask multiply, matching the reference where exp(-1e9 - max)
    underflows to exactly 0).  Only k-blocks j >= 4c are active, and
    diagonal blocks narrow to 128(d+1) q-columns.
    attn^T[dh|sum, q] accumulates matmul([V|1], p^T) over j in PSUM.
    The globally-masked last row (q=2047) is exactly uniform attention;
    recomputed exactly on the host (kept finite on-chip via [0|1]
    matmuls into columns 510:512).
    Normalization: broadcast the sums row to 64 partitions with a K=1
    ones matmul (bcs tiles live in the psO pool so the next chunk's
    scores never wait on a late reciprocal read of a psS bank), 64-lane
    reciprocal_approx_fast, multiply.  Odd heads go through a
    partition-shifting SBUF DMA into rows 64:128 of the pair tile.
  Phase C (fused per q-chunk): out_partial[s-blocks, :] = attn^T.T @
    Wo-rows, K=128 pair contraction; the first psum groups of each
    chunk are pre-opened with zero-add dummy matmuls (0-valued
    stationary) that fill the PE bubble while the norm chain drains and
    keep HAM from re-throttling; both 512-col halves land in one ob
    tile and ship as a single contiguous [128, 1024] DMA per s-block.
"""

import numpy as np

B, S, D, H, DH = 2, 2048, 1024, 16, 64
HPC = 4                 # heads per core
NCORES = 8
NPAIR = HPC // 2        # head pairs per core (2)
SBLK = S // 128         # 16 s/k blocks
NCH = S // 512          # 4 q-chunks of 512
CHUNKS = D // 128       # 8 contraction chunks of the model dim
START_DUMMIES = 20      # bf16 HAM warmers during the initial DMA fill
CWARM = 4               # zero-add matmuls pre-opening each C psum group

_CACHE = {}


def _build_nc(debug=False):
    import concourse.mybir as mybir
    from concourse import bacc, tile

    f32 = mybir.dt.float32
    bf16 = mybir.dt.bfloat16
    AF = mybir.ActivationFunctionType
    OP = mybir.AluOpType

    nc = bacc.Bacc("TRN2", target_bir_lowering=False)

    # x^T pre-quartered host-side: xq[n, p, c, s] = x^T[128c+p, 512n+s].
    # [n, p, c, s] order makes each quarter's DMA fully contiguous per
    # partition (8 KB descriptors instead of 1 KB).
    xq = nc.dram_tensor("xq", [NCH, 128, CHUNKS, 512], bf16,
                        kind="ExternalInput")
    # weights pre-packed host-side to the on-chip tile layouts
    wq = nc.dram_tensor("wq", [128, CHUNKS, HPC * DH], bf16,
                        kind="ExternalInput")
    wk = nc.dram_tensor("wk", [128, CHUNKS, HPC * DH], bf16,
                        kind="ExternalInput")
    wv = nc.dram_tensor("wv", [128, CHUNKS, HPC * DH], bf16,
                        kind="ExternalInput")
    wo = nc.dram_tensor("wo", [128, NPAIR, D], bf16, kind="ExternalInput")
    # per-pair q/k biases: [128, 4] cols = (q pair0, q pair1, k pair0, k pair1)
    bqk = nc.dram_tensor("bqk", [128, 2 * NPAIR], f32, kind="ExternalInput")
    # bv broadcast to all partitions host-side: [128, 256]
    bvb = nc.dram_tensor("bvb", [128, HPC * DH], f32, kind="ExternalInput")
    # diagonal-block causal masks: masks[k_local, d, f] = (128d + k_local > f)
    masks = nc.dram_tensor("masks", [128, 4, 512], bf16,
                           kind="ExternalInput")
    out = nc.dram_tensor("out", [S, D], f32, kind="ExternalOutput")
    if debug:
        qt_d = nc.dram_tensor("qt_d", [128, NPAIR, S], f32,
                              kind="ExternalOutput")
        kt_d = nc.dram_tensor("kt_d", [128, NPAIR, S], f32,
                              kind="ExternalOutput")
        vsb_d = nc.dram_tensor("vsb_d", [128, SBLK, HPC, DH + 1], f32,
                               kind="ExternalOutput")
        atn_d = nc.dram_tensor("atn_d", [128, NCH, NPAIR, 512], f32,
                               kind="ExternalOutput")
        psa_d = nc.dram_tensor("psa_d", [DH + 1, NCH, HPC, 512], f32,
                               kind="ExternalOutput")

    with tile.TileContext(nc) as tc:
        with (
            tc.tile_pool(name="pers", bufs=1) as pers,
        ):
            misc = pers
            # Q^T head pairs, two variants with the other head's rows
            # zeroed so score matmuls can contract K=128 (K=64 matmuls
            # register HAM activity poorly and can run at 1.2 GHz).
            # One tile [128, variant, pair, S] so diagonal blocks can run
            # both halves in a single matmul with a strided moving AP.
            qz2 = pers.tile([128, 2, NPAIR, S], bf16)
            kt = pers.tile([128, NPAIR, S], bf16)         # K^T head pairs
            vsb = pers.tile([128, SBLK, HPC, DH + 1], bf16)  # [V | 1]
            wo_t = pers.tile([128, NPAIR, D], bf16)       # Wo head pairs
            ones2 = misc.tile([128, 2], bf16)   # [0 | 1] columns
            onef = misc.tile([128, 2], f32)
            onesrow = misc.tile([DH + 1, DH], bf16)  # row 64 = ones
            bias_t = misc.tile([128, 2 * NPAIR], f32)
            bvb_t = misc.tile([128, HPC * DH], f32)
            mask_t = misc.tile([128, 4, 512], bf16)

            dmy_w = misc.tile([128, 128], bf16)
            dmy_z = misc.tile([128, 128], bf16)   # zero stationary: +0 adds
            dmy_m = misc.tile([128, 512], bf16)
            nc.gpsimd.memset(dmy_w[:], 0.25)
            nc.gpsimd.memset(dmy_z[:], 0.0)
            nc.gpsimd.memset(dmy_m[:], 0.25)
            nc.sync.dma_start(bias_t[:], bqk[:])
            nc.gpsimd.memset(onef[:, 0:1], 0.0)
            nc.gpsimd.memset(onef[:, 1:2], 1.0)
            nc.vector.tensor_copy(ones2[:], onef[:])  # bf16 [0|1]
            nc.vector.tensor_copy(
                onesrow[DH:DH + 1, :],
                onef[DH:DH + 1, 1:2].to_broadcast((1, DH)))
            # ones column of [V|1] for every (sblk, head)
            nc.vector.tensor_copy(
                vsb[:, :, :, DH:DH + 1],
                onef[:, 1:2].to_broadcast((128, SBLK, HPC, 1)))
            nc.vector.tensor_copy(
                qz2[64:128, 0, :, :],
                onef[64:128, 0:1].to_broadcast((64, NPAIR, S)))
            nc.vector.tensor_copy(
                qz2[0:64, 1, :, :],
                onef[0:64, 0:1].to_broadcast((64, NPAIR, S)))

            # Pre-computed exp(score) tiles: chunk0's j=0..11 and chunk1's
            # diagonal j=4..7 are scored+exp'd DURING phase A (interleaved
            # between projection groups) so the ACT engine's exp backlog
            # starts draining while the PE is projection-bound.  Without
            # this, chunk0 is ACT-bound (41us of exp vs 29us of matmul).
            PRE = ([(0, j, j) for j in range(12)]
                   + [(1, j, 8 + j) for j in range(4, 8)])
            # slot-reuse prescores emitted DURING phase B (slots freed by
            # chunk0's early attnV reads): ch2's diag into slots 0..3,
            # ch3's diag into slots 8..11.  Emission points (ch, idx)
            # are placed AFTER the attnV that frees the slot, so the
            # ACT exp never inverts the PE FIFO order (deadlock-safe).
            PRE_B = {}
            pre_map = {(c_, j_): s_ for c_, j_, s_ in PRE}
            for blks in PRE_B.values():
                for c_, j_, s_ in blks:
                    pre_map[(c_, j_)] = s_
            ptbuf = pers.tile([128, 16, HPC, 512], bf16)

            with (
                tc.tile_pool(name="psS", bufs=2, space="PSUM") as psS,
            ):
                def emit_score_block(ch, j, pts4):
                    """scores + exp + causal mask for one (chunk, k-block):
                    4 head tiles of exp(score^T/8) land in pts4."""
                    d = j - 4 * ch
                    W = 128 * (d + 1) if d < 4 else 512
                    for pair in range(NPAIR):
                        merged = d < 2  # both halves in one matmul
                        if merged:
                            pss = psS.tile([128, 512], f32, tag="pss",
                                           name="pss")
                            nc.tensor.matmul(
                                pss[:].rearrange(
                                    "p (v w) -> p v w", v=2)[:, :, 0:W],
                                kt[:, pair, 128 * j:128 * j + 128],
                                qz2[:, :, pair, 512 * ch:512 * ch + W],
                                start=True, stop=True)
                        for half in range(2):
                            h = 2 * pair + half
                            if not merged:
                                pss = psS.tile([128, 512], f32,
                                               tag="pss", name="pss")
                                nc.tensor.matmul(
                                    pss[:, 0:W],
                                    kt[:, pair, 128 * j:128 * j + 128],
                                    qz2[:, half, pair,
                                        512 * ch:512 * ch + W],
                                    start=True, stop=True)
                                src_ap = pss[:, 0:W]
                            else:
                                src_ap = pss[:, 256 * half:256 * half + W]
                            nc.scalar.activation(pts4[h][:, 0:W], src_ap,
                                                 AF.Exp, scale=0.125)
                            if d < 4:
                                # zero where k <= q (DVE mask multiply)
                                nc.vector.tensor_tensor(
                                    pts4[h][:, 0:W], pts4[h][:, 0:W],
                                    mask_t[:, d, 0:W], op=OP.mult)

                # ---------------- Phase A: projections ----------------
                ctxA = nc.named_scope("phaseA"); ctxA.__enter__()
                with (
                    tc.tile_pool(name="wts", bufs=1) as wts,
                    tc.tile_pool(name="psA", bufs=2,
                                 space="PSUM") as psA,
                ):
                    psV = psA
                    psW = psA
                    dmy_ps = psW.tile([128, 512], f32, name="dmy_ps",
                                      tag="dmy", bufs=1)
                    # quarter-major x so each quarter's DMA is contiguous
                    xtr = wts.tile([128, NCH, CHUNKS, 512], bf16)
                    wq_t = wts.tile([128, CHUNKS, HPC * DH], bf16,
                                    tag="wq")
                    wk_t = wts.tile([128, CHUNKS, HPC * DH], bf16,
                                    tag="wk")
                    wv_t = wts.tile([128, CHUNKS, HPC * DH], bf16,
                                    tag="wv")

                    # DMA priority: the DGE round-robins descriptors of
                    # every queued transfer, so simply emitting the
                    # critical loads first is NOT enough — later loads
                    # steal bandwidth.  Gate each non-critical load on a
                    # 1-element marker write into its destination whose
                    # source is a projection output: the DMA then can't
                    # start until the pipeline actually needs it.
                    nc.sync.dma_start(wq_t[:], wq[:])
                    nc.sync.dma_start(mask_t[:], masks[:])
                    nc.sync.dma_start(bvb_t[:], bvb[:])
                    # x quarter 0 in chunk-pair sub-DMAs so the first
                    # projection group can start on partial data
                    for cp in range(4):
                        nc.scalar.dma_start(xtr[:, 0, 2 * cp:2 * cp + 2],
                                            xq[0, :, 2 * cp:2 * cp + 2])

                    def gated(engine, marker_dst, marker_src, dst_ap,
                              src_ap):
                        nc.vector.tensor_copy(marker_dst, marker_src)
                        engine.dma_start(dst_ap, src_ap)

                    # wk/wv unlock once the first Q evac lands (~3us in)
                    gated(nc.sync, wk_t[0:1, 0:1, 0:1],
                          qz2[0:1, 0, 0, 0:1], wk_t[:], wk[:])
                    gated(nc.scalar, wv_t[0:1, 0:1, 0:1],
                          qz2[0:1, 0, 0, 0:1], wv_t[:], wv[:])
                    # x quarter n unlocks on Q(p0, n-1)'s evac
                    gated(nc.sync, xtr[0:1, 1, 0:1, 0:1],
                          qz2[0:1, 0, 0, 0:1], xtr[:, 1], xq[1])
                    gated(nc.scalar, xtr[0:1, 2, 0:1, 0:1],
                          qz2[0:1, 0, 0, 512:513], xtr[:, 2], xq[2])
                    gated(nc.sync, xtr[0:1, 3, 0:1, 0:1],
                          qz2[0:1, 0, 0, 1024:1025], xtr[:, 3], xq[3])
                    # wo is first needed by chunk0's phase C, much later
                    gated(nc.scalar, wo_t[0:1, 0:1, 0:1],
                          qz2[0:1, 0, 0, 1536:1537], wo_t[:], wo[:])
                    # Burn the HAM ramp while the first loads land: ~3.4us
                    # of dummy bf16 matmuls brings the PE to 2.4 GHz right
                    # as the first projection group's inputs arrive.
                    for _ in range(START_DUMMIES):
                        nc.tensor.matmul(dmy_ps[:], dmy_w[:], dmy_m[:],
                                         start=True, stop=True)

                    def proj_group(dsts, p, n):
                        w_tile = wq_t if dsts == "q" else wk_t
                        bcol0 = 0 if dsts == "q" else NPAIR
                        ps = psA.tile([128, 512], f32, name="ps", tag="ps")
                        for c in range(CHUNKS):
                            nc.tensor.matmul(
                                ps[:],
                                w_tile[:, c, 128 * p:128 * p + 128],
                                xtr[:, n, c, :],
                                start=(c == 0), stop=(c == CHUNKS - 1))
                        # evacuate + add per-partition bias (dh rows)
                        sl = slice(512 * n, 512 * n + 512)
                        bias = bias_t[:, bcol0 + p:bcol0 + p + 1]
                        if dsts == "k":
                            nc.scalar.activation(
                                kt[:, p, sl], ps[:], AF.Identity,
                                bias=bias)
                        else:
                            nc.scalar.activation(
                                qz2[0:64, 0, p, sl], ps[0:64, :],
                                AF.Identity, bias=bias[0:64, :])
                            nc.scalar.activation(
                                qz2[64:128, 1, p, sl], ps[64:128, :],
                                AF.Identity, bias=bias[64:128, :])

                    # n-major so each quarter's Q/K/V needs only x quarter
                    # n; the PE starts on quarter 0 while 1-3 stream in.
                    # Pre-score blocks are interleaved between projection
                    # groups of the NEXT quarter (their K/Q deps are done).
                    pre_by_n = {1: PRE[0:4], 2: PRE[4:8],
                                3: PRE[8:16]}
                    for n in range(NCH):
                        pre_list = list(pre_by_n.get(n, []))
                        pre_pts = [8, 8, 8, 8][n]

                        def pre_step(k=1):
                            for _ in range(k):
                                if pre_list:
                                    c_, j_, s_ = pre_list.pop(0)
                                    emit_score_block(
                                        c_, j_,
                                        [ptbuf[:, s_, h_, :]
                                         for h_ in range(HPC)])

                        for p in range(NPAIR):
                            proj_group("q", p, n)
                            pre_step()
                        for p in range(NPAIR):
                            proj_group("k", p, n)
                            pre_step()
                        for sb in range(4 * n, 4 * n + 4):
                            k = sb - 4 * n
                            ps = psV.tile([128, HPC * DH], f32,
                                          name="psv", tag="psv",
                                          bufs=2)
                            for c in range(CHUNKS):
                                nc.tensor.matmul(
                                    ps[:],
                                    xtr[:, n, c, 128 * k:128 * k + 128],
                                    wv_t[:, c, :],
                                    start=(c == 0),
                                    stop=(c == CHUNKS - 1))
                            nc.vector.tensor_tensor(
                                vsb[:, sb, :, 0:DH],
                                ps[:].rearrange("p (h d) -> p h d", h=HPC),
                                bvb_t[:].rearrange("p (h d) -> p h d",
                                                   h=HPC),
                                op=OP.add)
                            pre_step(2 if n == 3 else 1)

                ctxA.__exit__(None, None, None)
                # ------------- Phase B + fused C, per q-chunk -------------
                with (
                    tc.tile_pool(name="wrk", bufs=2) as wrk,
                    tc.tile_pool(name="psB", bufs=1,
                                 space="PSUM") as psB,
                ):
                    srowp = rcpp = toddp = obp = wrk
                    psAt = psO = psB
                    for ch in range(NCH):
                        ctxB = nc.named_scope(f"chunk{ch}")
                        ctxB.__enter__()
                        js = list(range(4 * ch, SBLK))
                        psa = [psAt.tile([DH + 1, 512], f32,
                                         tag=f"psa{h}", name=f"psa{h}")
                               for h in range(HPC)]
                        for idx, j in enumerate(js):
                            d = j - 4 * ch
                            W = 128 * (d + 1) if d < 4 else 512
                            slot = pre_map.get((ch, j))
                            if slot is not None:
                                pts = [ptbuf[:, slot, h, :]
                                       for h in range(HPC)]
                            else:
                                pts = [pers.tile([128, 512], bf16,
                                                 name="pt", tag="pt",
                                                 bufs=6)
                                       for _ in range(HPC)]
                                emit_score_block(ch, j, pts)
                            last = (idx == len(js) - 1) and ch < 3
                            for h in range(HPC):
                                nc.tensor.matmul(
                                    psa[h][:, 0:W], vsb[:, j, h, :],
                                    pts[h][:, 0:W],
                                    start=(idx == 0), stop=last,
                                    skip_group_check=(ch == 3))
                            for c_, j_, s_ in PRE_B.get((ch, idx), []):
                                emit_score_block(
                                    c_, j_,
                                    [ptbuf[:, s_, h_, :]
                                     for h_ in range(HPC)])
                        if ch == 3:
                            # last global row q=2047 is fully masked; its
                            # exact value (uniform attention = mean(V)@Wo)
                            # is recomputed on the host.  Keep column
                            # 511's denominator finite (one [0|1]-column
                            # matmul) to avoid Inf/NaN noise.
                            for h in range(HPC):
                                nc.tensor.matmul(
                                    psa[h][:, 510:512],
                                    vsb[:, 0, h, :], ones2[:],
                                    start=False, stop=True)
                        # normalize: attn^T rows / sums row.  Broadcast
                        # the sums row via a K=1 ones matmul, 64-lane
                        # approx reciprocal, then multiply.  Odd heads go
                        # through a SBUF tile and a partition-shifting DMA
                        # into rows 64:128 of the pair tile so phase C
                        # contracts K=128.  bcs tiles allocate from the
                        # psO pool (NOT psS) so the next chunk's first
                        # score matmul never waits on a late reciprocal
                        # read of a psS bank.
                        atn = pers.tile([128, NPAIR, 512], bf16,
                                        name="atn", tag="atn", bufs=2)
                        srows = []
                        for h in range(HPC):
                            srow = srowp.tile([DH + 1, 512], bf16)
                            nc.scalar.copy(srow[DH:DH + 1, :],
                                           psa[h][DH:DH + 1, :])
                            srows.append(srow)
                        rcps = []
                        for h in range(HPC):
                            bcs = psO.tile([128, 512], f32, name="bcs",
                                           tag="po", bufs=2)
                            # zero-add filler first: depends only on the
                            # psum bank, so the PE stays busy (and HAM
                            # stays hot) while the srow copy lands
                            nc.tensor.matmul(bcs[0:64, :],
                                             dmy_z[:, 0:64], dmy_m[:],
                                             start=True, stop=False)
                            nc.tensor.matmul(bcs[0:64, :],
                                             onesrow[DH:DH + 1, :],
                                             srows[h][DH:DH + 1, :],
                                             start=False, stop=True)
                            rcp = rcpp.tile([64, 512], f32)
                            nc.vector.reciprocal_approx_fast(rcp[:],
                                                             bcs[0:64, :])
                            rcps.append(rcp)
                        for h in range(HPC):
                            pair, half = h // 2, h % 2
                            if half == 0:
                                nc.vector.tensor_tensor(
                                    atn[0:64, pair, :], psa[h][0:DH, :],
                                    rcps[h][:], op=OP.mult)
                            else:
                                todd = toddp.tile([64, 512], bf16)
                                nc.vector.tensor_tensor(
                                    todd[:], psa[h][0:DH, :], rcps[h][:],
                                    op=OP.mult)
                                nc.sync.dma_start(atn[64:128, pair, :],
                                                  todd[:])

                        # fused phase C for this chunk's 4 s-blocks.  The
                        # first two psum groups are pre-opened with
                        # zero-add dummy matmuls: they depend only on free
                        # psO banks, so they fill the PE bubble while the
                        # norm chain (copy -> bcast -> rcp -> mult ->
                        # shift DMA) drains, and keep HAM warm.
                        for k in range(4):
                            sb = 4 * ch + k
                            ob = obp.tile([128, 2, 512], f32, name="ob",
                                          tag="ob", bufs=2)
                            for n in range(2):
                                ps = psO.tile([128, 512], f32,
                                              name="po", tag="po",
                                              bufs=2)
                                warm = (CWARM if k == 0 else
                                        (2 if ch == 3 else 0))
                                for w in range(warm):
                                    nc.tensor.matmul(
                                        ps[:], dmy_z[:], dmy_m[:],
                                        start=(w == 0), stop=False)
                                for p in range(NPAIR):
                                    nc.tensor.matmul(
                                        ps[:],
                                        atn[:, p, 128 * k:128 * k + 128],
                                        wo_t[:, p, 512 * n:512 * n + 512],
                                        start=(warm == 0 and p == 0),
                                        stop=(p == NPAIR - 1))
                                if n == 0:
                                    nc.scalar.copy(ob[:, 0, :], ps[:])
                                else:
                                    nc.vector.tensor_copy(ob[:, 1, :],
                                                          ps[:])
                            if ch == 3 and k == 3:
                                # final store: split across both rings so
                                # the drain isn\'t one serial 512KB DMA
                                nc.sync.dma_start(
                                    out[128 * sb:128 * sb + 128, 0:512],
                                    ob[:, 0, :])
                                nc.scalar.dma_start(
                                    out[128 * sb:128 * sb + 128,
                                        512:1024], ob[:, 1, :])
                            else:
                                nc.sync.dma_start(
                                    out[128 * sb:128 * sb + 128, :],
                                    ob[:].rearrange("p a b -> p (a b)"))
                        ctxB.__exit__(None, None, None)

    nc.finalize()
    return nc


def _prep_in_maps(inputs, Wq, bq, Wk, bk, Wv, bv, Wo, bo):
    import ml_dtypes
    bf = ml_dtypes.bfloat16
    in_maps = []
    # xq[n, p, c, s] = x^T[128c+p, 512n+s]: each (quarter, partition) is
    # a contiguous 8KB run, so the DMA gets full-size descriptors
    xqs = []
    for b in range(B):
        xT = np.ascontiguousarray(inputs[b].T).astype(bf)
        xqs.append(np.ascontiguousarray(
            xT.reshape(CHUNKS, 128, NCH, 512).transpose(2, 1, 0, 3)))
    kk = np.arange(128)[:, None, None]
    dd = np.arange(4)[None, :, None]
    ff = np.arange(512)[None, None, :]
    masks = ((128 * dd + kk) > ff).astype(bf)
    for core in range(NCORES):
        b = core // (NCORES // B)
        g = core % (NCORES // B)
        cols = slice(g * HPC * DH, (g + 1) * HPC * DH)
        bq_c = bq[cols].reshape(NPAIR, 128).T          # [128, 2]
        bk_c = bk[cols].reshape(NPAIR, 128).T
        bqk_c = np.ascontiguousarray(
            np.concatenate([bq_c, bk_c], axis=1), dtype=np.float32)
        bvb_c = np.ascontiguousarray(
            np.broadcast_to(bv[cols][None, :], (128, HPC * DH)),
            dtype=np.float32)

        def pack_w(w):  # [D, HPC*DH] -> [128, CHUNKS, HPC*DH]
            return np.ascontiguousarray(
                w.astype(bf).reshape(CHUNKS, 128, HPC * DH)
                .transpose(1, 0, 2))

        wo_c = np.ascontiguousarray(
            Wo[cols, :].astype(bf).reshape(NPAIR, 128, D)
            .transpose(1, 0, 2))
        in_maps.append({
            "xq": xqs[b],
            "wq": pack_w(np.ascontiguousarray(Wq[:, cols])),
            "wk": pack_w(np.ascontiguousarray(Wk[:, cols])),
            "wv": pack_w(np.ascontiguousarray(Wv[:, cols])),
            "wo": wo_c,
            "bqk": bqk_c,
            "bvb": bvb_c,
            "masks": masks,
        })
    return in_maps


def kernel(inputs, Wq, bq, Wk, bk, Wv, bv, Wo, bo, _want_results=False,
           **_run_kwargs):
    from concourse.bass_utils import run_bass_kernel_spmd

    inputs = np.asarray(inputs, dtype=np.float32)
    Wq, bq = np.asarray(Wq, np.float32), np.asarray(bq, np.float32)
    Wk, bk = np.asarray(Wk, np.float32), np.asarray(bk, np.float32)
    Wv, bv = np.asarray(Wv, np.float32), np.asarray(bv, np.float32)
    Wo, bo = np.asarray(Wo, np.float32), np.asarray(bo, np.float32)

    if "nc" not in _CACHE:
        _CACHE["nc"] = _build_nc()
    nc = _CACHE["nc"]

    in_maps = _prep_in_maps(inputs, Wq, bq, Wk, bk, Wv, bv, Wo, bo)
    res = run_bass_kernel_spmd(nc, in_maps, core_ids=list(range(NCORES)),
                               **_run_kwargs)

    out = np.zeros((B, S, D), dtype=np.float32)
    for core in range(NCORES):
        b = core // (NCORES // B)
        out[b] += res.results[core]["out"]
    out += bo[None, None, :]
    # exact last row (fully masked -> uniform attention = mean(V) @ Wo)
    for b in range(B):
        v_mean = inputs[b].mean(axis=0) @ Wv + bv
        out[b, S - 1, :] = v_mean @ Wo + bo
    if _want_results:
        return out, res
    return out



# revision 43
# speedup vs baseline: 1.0200x; 1.0016x over previous
"""Causal self-attention (with the reference's inverted mask) on 8 TRN2
NeuronCores.

Problem (hardcoded): B=2, S=2048, D=1024, H=16 heads, head_dim=64, fp32.
  q/k/v = x @ W* + b*;  score = q k^T / 8;  score += tril(ones)*(-1e9)
  (inverted causal mask: the LOWER triangle incl. diagonal is masked, so
  softmax attends strictly to k > q; row q=S-1 is fully masked and its
  softmax is exactly uniform, since all its masked inputs round to exactly
  -1e9 in fp32);  out = softmax(score) @ v @ Wo + bo.

Sharding: core c handles batch b = c//4 and heads [4*(c%4), 4*(c%4)+4).
Each core computes a partial output (its 4 heads' slice of attn @ Wo);
the host sums 4 partials per batch and adds bo.

Per-core kernel (all matmul operands bf16, fp32 PSUM accumulation:
~4.6e-3 rel err vs the fp32 reference, well under the 2e-2 gate; bf16
registers HAM activity so the PE sustains 2.4 GHz instead of the 1.2
GHz mid p-state that f32r was stuck at — this halved the kernel time):
  Phase A: QT/KT = W^T x^T in [dh, s] layout (head pairs packed to 128
    partitions), V in [s, dh] layout with an extra ones column per head
    ([V | 1]) so one matmul later yields both the attn numerator and the
    softmax denominator.  Projections run n-major (per 512-col x
    quarter) so the PE starts as soon as quarter 0 lands; x is loaded
    quarter-major with host-prepacked [n,p,c,s] layout (8KB contiguous
    DMA descriptors); non-critical loads are gated on 1-element marker
    writes so the DGE can't steal bandwidth from the critical prefix.
    START_DUMMIES bf16 matmuls burn the HAM ramp during the DMA fill.
    Chunk0's k-blocks j=0..11 and chunk1's diagonal j=4..7 are ALSO
    scored+exp'd here, interleaved between projection groups into the
    persistent ptbuf: without this, chunk0 is ACT-bound (41us of exp vs
    29us of PE), since exp on ACT costs ~2.2us per off-diag k-block vs
    1.7us of PE work.
  Phase B (per q-chunk of 512): scores computed TRANSPOSED,
    s^T[k, q] = K^T Q per (head, k-block j), so softmax needs no
    max-subtraction and no transposes of the probability matrix:
    p^T = exp(s^T/8) in bf16 (masked entries zero-filled via a bf16 DVE
    mask multiply, matching the reference where exp(-1e9 - max)
    underflows to exactly 0).  Only k-blocks j >= 4c are active, and
    diagonal blocks narrow to 128(d+1) q-columns.
    attn^T[dh|sum, q] accumulates matmul([V|1], p^T) over j in PSUM.
    The globally-masked last row (q=2047) is exactly uniform attention;
    recomputed exactly on the host (kept finite on-chip via [0|1]
    matmuls into columns 510:512).
    Normalization: broadcast the sums row to 64 partitions with a K=1
    ones matmul (bcs tiles live in the psO pool so the next chunk's
    scores never wait on a late reciprocal read of a psS bank), 64-lane
    reciprocal_approx_fast, multiply.  Odd heads go through a
    partition-shifting SBUF DMA into rows 64:128 of the pair tile.
  Phase C (fused per q-chunk): out_partial[s-blocks, :] = attn^T.T @
    Wo-rows, K=128 pair contraction; the first psum groups of each
    chunk are pre-opened with zero-add dummy matmuls (0-valued
    stationary) that fill the PE bubble while the norm chain drains and
    keep HAM from re-throttling; both 512-col halves land in one ob
    tile and ship as a single contiguous [128, 1024] DMA per s-block.
"""

import numpy as np

B, S, D, H, DH = 2, 2048, 1024, 16, 64
HPC = 4                 # heads per core
NCORES = 8
NPAIR = HPC // 2        # head pairs per core (2)
SBLK = S // 128         # 16 s/k blocks
NCH = S // 512          # 4 q-chunks of 512
CHUNKS = D // 128       # 8 contraction chunks of the model dim
START_DUMMIES = 20      # bf16 HAM warmers during the initial DMA fill
CWARM = 4               # zero-add matmuls pre-opening each C psum group

_CACHE = {}


def _build_nc(debug=False):
    import concourse.mybir as mybir
    from concourse import bacc, tile

    f32 = mybir.dt.float32
    bf16 = mybir.dt.bfloat16
    AF = mybir.ActivationFunctionType
    OP = mybir.AluOpType

    nc = bacc.Bacc("TRN2", target_bir_lowering=False)

    # x^T pre-quartered host-side: xq[n, p, c, s] = x^T[128c+p, 512n+s].
    # [n, p, c, s] order makes each quarter's DMA fully contiguous per
    # partition (8 KB descriptors instead of 1 KB).
    xq = nc.dram_tensor("xq", [NCH, 128, CHUNKS, 512], bf16,
                        kind="ExternalInput")
    # weights pre-packed host-side to the on-chip tile layouts
    wq = nc.dram_tensor("wq", [128, CHUNKS, HPC * DH], bf16,
                        kind="ExternalInput")
    wk = nc.dram_tensor("wk", [128, CHUNKS, HPC * DH], bf16,
                        kind="ExternalInput")
    wv = nc.dram_tensor("wv", [128, CHUNKS, HPC * DH], bf16,
                        kind="ExternalInput")
    wo = nc.dram_tensor("wo", [128, NPAIR, D], bf16, kind="ExternalInput")
    # per-pair q/k biases: [128, 4] cols = (q pair0, q pair1, k pair0, k pair1)
    bqk = nc.dram_tensor("bqk", [128, 2 * NPAIR], f32, kind="ExternalInput")
    # bv broadcast to all partitions host-side: [128, 256]
    bvb = nc.dram_tensor("bvb", [128, HPC * DH], f32, kind="ExternalInput")
    # diagonal-block causal masks: masks[k_local, d, f] = (128d + k_local > f)
    masks = nc.dram_tensor("masks", [128, 4, 512], bf16,
                           kind="ExternalInput")
    out = nc.dram_tensor("out", [S, D], f32, kind="ExternalOutput")
    if debug:
        qt_d = nc.dram_tensor("qt_d", [128, NPAIR, S], f32,
                              kind="ExternalOutput")
        kt_d = nc.dram_tensor("kt_d", [128, NPAIR, S], f32,
                              kind="ExternalOutput")
        vsb_d = nc.dram_tensor("vsb_d", [128, SBLK, HPC, DH + 1], f32,
                               kind="ExternalOutput")
        atn_d = nc.dram_tensor("atn_d", [128, NCH, NPAIR, 512], f32,
                               kind="ExternalOutput")
        psa_d = nc.dram_tensor("psa_d", [DH + 1, NCH, HPC, 512], f32,
                               kind="ExternalOutput")

    with tile.TileContext(nc) as tc:
        with (
            tc.tile_pool(name="pers", bufs=1) as pers,
        ):
            misc = pers
            # Q^T head pairs, two variants with the other head's rows
            # zeroed so score matmuls can contract K=128 (K=64 matmuls
            # register HAM activity poorly and can run at 1.2 GHz).
            # One tile [128, variant, pair, S] so diagonal blocks can run
            # both halves in a single matmul with a strided moving AP.
            qz2 = pers.tile([128, 2, NPAIR, S], bf16)
            kt = pers.tile([128, NPAIR, S], bf16)         # K^T head pairs
            vsb = pers.tile([128, SBLK, HPC, DH + 1], bf16)  # [V | 1]
            wo_t = pers.tile([128, NPAIR, D], bf16)       # Wo head pairs
            ones2 = misc.tile([128, 2], bf16)   # [0 | 1] columns
            onef = misc.tile([128, 2], f32)
            onesrow = misc.tile([DH + 1, DH], bf16)  # row 64 = ones
            bias_t = misc.tile([128, 2 * NPAIR], f32)
            bvb_t = misc.tile([128, HPC * DH], f32)
            mask_t = misc.tile([128, 4, 512], bf16)

            dmy_w = misc.tile([128, 128], bf16)
            dmy_z = misc.tile([128, 128], bf16)   # zero stationary: +0 adds
            dmy_m = misc.tile([128, 512], bf16)
            nc.gpsimd.memset(dmy_w[:], 0.25)
            nc.gpsimd.memset(dmy_z[:], 0.0)
            nc.gpsimd.memset(dmy_m[:], 0.25)
            nc.sync.dma_start(bias_t[:], bqk[:])
            nc.gpsimd.memset(onef[:, 0:1], 0.0)
            nc.gpsimd.memset(onef[:, 1:2], 1.0)
            nc.vector.tensor_copy(ones2[:], onef[:])  # bf16 [0|1]
            nc.vector.tensor_copy(
                onesrow[DH:DH + 1, :],
                onef[DH:DH + 1, 1:2].to_broadcast((1, DH)))
            # ones column of [V|1] for every (sblk, head)
            nc.vector.tensor_copy(
                vsb[:, :, :, DH:DH + 1],
                onef[:, 1:2].to_broadcast((128, SBLK, HPC, 1)))
            nc.vector.tensor_copy(
                qz2[64:128, 0, :, :],
                onef[64:128, 0:1].to_broadcast((64, NPAIR, S)))
            nc.vector.tensor_copy(
                qz2[0:64, 1, :, :],
                onef[0:64, 0:1].to_broadcast((64, NPAIR, S)))

            # Pre-computed exp(score) tiles: chunk0's j=0..11 and chunk1's
            # diagonal j=4..7 are scored+exp'd DURING phase A (interleaved
            # between projection groups) so the ACT engine's exp backlog
            # starts draining while the PE is projection-bound.  Without
            # this, chunk0 is ACT-bound (41us of exp vs 29us of matmul).
            PRE = ([(0, j, j) for j in range(12)]
                   + [(1, j, 8 + j) for j in range(4, 8)])
            # slot-reuse prescores emitted DURING phase B (slots freed by
            # chunk0's early attnV reads): ch2's diag into slots 0..3,
            # ch3's diag into slots 8..11.  Emission points (ch, idx)
            # are placed AFTER the attnV that frees the slot, so the
            # ACT exp never inverts the PE FIFO order (deadlock-safe).
            PRE_B = {}
            pre_map = {(c_, j_): s_ for c_, j_, s_ in PRE}
            for blks in PRE_B.values():
                for c_, j_, s_ in blks:
                    pre_map[(c_, j_)] = s_
            ptbuf = pers.tile([128, 16, HPC, 512], bf16)

            with (
                tc.tile_pool(name="psS", bufs=2, space="PSUM") as psS,
            ):
                def emit_score_block(ch, j, pts4):
                    """scores + exp + causal mask for one (chunk, k-block):
                    4 head tiles of exp(score^T/8) land in pts4."""
                    d = j - 4 * ch
                    W = 128 * (d + 1) if d < 4 else 512
                    for pair in range(NPAIR):
                        merged = d < 2  # both halves in one matmul
                        if merged:
                            pss = psS.tile([128, 512], f32, tag="pss",
                                           name="pss")
                            nc.tensor.matmul(
                                pss[:].rearrange(
                                    "p (v w) -> p v w", v=2)[:, :, 0:W],
                                kt[:, pair, 128 * j:128 * j + 128],
                                qz2[:, :, pair, 512 * ch:512 * ch + W],
                                start=True, stop=True)
                        for half in range(2):
                            h = 2 * pair + half
                            if not merged:
                                pss = psS.tile([128, 512], f32,
                                               tag="pss", name="pss")
                                nc.tensor.matmul(
                                    pss[:, 0:W],
                                    kt[:, pair, 128 * j:128 * j + 128],
                                    qz2[:, half, pair,
                                        512 * ch:512 * ch + W],
                                    start=True, stop=True)
                                src_ap = pss[:, 0:W]
                            else:
                                src_ap = pss[:, 256 * half:256 * half + W]
                            nc.scalar.activation(pts4[h][:, 0:W], src_ap,
                                                 AF.Exp, scale=0.125)
                            if d < 4:
                                # zero where k <= q (DVE mask multiply)
                                nc.vector.tensor_tensor(
                                    pts4[h][:, 0:W], pts4[h][:, 0:W],
                                    mask_t[:, d, 0:W], op=OP.mult)

                # ---------------- Phase A: projections ----------------
                ctxA = nc.named_scope("phaseA"); ctxA.__enter__()
                with (
                    tc.tile_pool(name="wts", bufs=1) as wts,
                    tc.tile_pool(name="psA", bufs=2,
                                 space="PSUM") as psA,
                ):
                    psV = psA
                    psW = psA
                    dmy_ps = psW.tile([128, 512], f32, name="dmy_ps",
                                      tag="dmy", bufs=1)
                    # quarter-major x so each quarter's DMA is contiguous
                    xtr = wts.tile([128, NCH, CHUNKS, 512], bf16)
                    wq_t = wts.tile([128, CHUNKS, HPC * DH], bf16,
                                    tag="wq")
                    wk_t = wts.tile([128, CHUNKS, HPC * DH], bf16,
                                    tag="wk")
                    wv_t = wts.tile([128, CHUNKS, HPC * DH], bf16,
                                    tag="wv")

                    # DMA priority: the DGE round-robins descriptors of
                    # every queued transfer, so simply emitting the
                    # critical loads first is NOT enough — later loads
                    # steal bandwidth.  Gate each non-critical load on a
                    # 1-element marker write into its destination whose
                    # source is a projection output: the DMA then can't
                    # start until the pipeline actually needs it.
                    nc.sync.dma_start(wq_t[:], wq[:])
                    nc.sync.dma_start(mask_t[:], masks[:])
                    nc.sync.dma_start(bvb_t[:], bvb[:])
                    # x quarter 0 in chunk-pair sub-DMAs so the first
                    # projection group can start on partial data
                    for cp in range(4):
                        nc.scalar.dma_start(xtr[:, 0, 2 * cp:2 * cp + 2],
                                            xq[0, :, 2 * cp:2 * cp + 2])

                    def gated(engine, marker_dst, marker_src, dst_ap,
                              src_ap):
                        nc.vector.tensor_copy(marker_dst, marker_src)
                        engine.dma_start(dst_ap, src_ap)

                    # wk/wv unlock once the first Q evac lands (~3us in)
                    gated(nc.sync, wk_t[0:1, 0:1, 0:1],
                          qz2[0:1, 0, 0, 0:1], wk_t[:], wk[:])
                    gated(nc.scalar, wv_t[0:1, 0:1, 0:1],
                          qz2[0:1, 0, 0, 0:1], wv_t[:], wv[:])
                    # x quarter n unlocks on Q(p0, n-1)'s evac
                    gated(nc.sync, xtr[0:1, 1, 0:1, 0:1],
                          qz2[0:1, 0, 0, 0:1], xtr[:, 1], xq[1])
                    gated(nc.scalar, xtr[0:1, 2, 0:1, 0:1],
                          qz2[0:1, 0, 0, 512:513], xtr[:, 2], xq[2])
                    gated(nc.sync, xtr[0:1, 3, 0:1, 0:1],
                          qz2[0:1, 0, 0, 1024:1025], xtr[:, 3], xq[3])
                    # wo is first needed by chunk0's phase C, much later
                    gated(nc.scalar, wo_t[0:1, 0:1, 0:1],
                          qz2[0:1, 0, 0, 1536:1537], wo_t[:], wo[:])
                    # Burn the HAM ramp while the first loads land: ~3.4us
                    # of dummy bf16 matmuls brings the PE to 2.4 GHz right
                    # as the first projection group's inputs arrive.
                    for _ in range(START_DUMMIES):
                        nc.tensor.matmul(dmy_ps[:], dmy_w[:], dmy_m[:],
                                         start=True, stop=True)

                    def proj_group(dsts, p, n):
                        w_tile = wq_t if dsts == "q" else wk_t
                        bcol0 = 0 if dsts == "q" else NPAIR
                        ps = psA.tile([128, 512], f32, name="ps", tag="ps")
                        for c in range(CHUNKS):
                            nc.tensor.matmul(
                                ps[:],
                                w_tile[:, c, 128 * p:128 * p + 128],
                                xtr[:, n, c, :],
                                start=(c == 0), stop=(c == CHUNKS - 1))
                        # evacuate + add per-partition bias (dh rows)
                        sl = slice(512 * n, 512 * n + 512)
                        bias = bias_t[:, bcol0 + p:bcol0 + p + 1]
                        if dsts == "k":
                            nc.scalar.activation(
                                kt[:, p, sl], ps[:], AF.Identity,
                                bias=bias)
                        else:
                            nc.scalar.activation(
                                qz2[0:64, 0, p, sl], ps[0:64, :],
                                AF.Identity, bias=bias[0:64, :])
                            nc.scalar.activation(
                                qz2[64:128, 1, p, sl], ps[64:128, :],
                                AF.Identity, bias=bias[64:128, :])

                    # n-major so each quarter's Q/K/V needs only x quarter
                    # n; the PE starts on quarter 0 while 1-3 stream in.
                    # Pre-score blocks are interleaved between projection
                    # groups of the NEXT quarter (their K/Q deps are done).
                    pre_by_n = {1: PRE[0:4], 2: PRE[4:8],
                                3: PRE[8:16]}
                    for n in range(NCH):
                        pre_list = list(pre_by_n.get(n, []))
                        pre_pts = [8, 8, 8, 8][n]

                        def pre_step(k=1):
                            for _ in range(k):
                                if pre_list:
                                    c_, j_, s_ = pre_list.pop(0)
                                    emit_score_block(
                                        c_, j_,
                                        [ptbuf[:, s_, h_, :]
                                         for h_ in range(HPC)])

                        for p in range(NPAIR):
                            proj_group("q", p, n)
                            pre_step()
                        for p in range(NPAIR):
                            proj_group("k", p, n)
                            pre_step()
                        for sb in range(4 * n, 4 * n + 4):
                            k = sb - 4 * n
                            ps = psV.tile([128, HPC * DH], f32,
                                          name="psv", tag="psv",
                                          bufs=2)
                            for c in range(CHUNKS):
                                nc.tensor.matmul(
                                    ps[:],
                                    xtr[:, n, c, 128 * k:128 * k + 128],
                                    wv_t[:, c, :],
                                    start=(c == 0),
                                    stop=(c == CHUNKS - 1))
                            nc.vector.tensor_tensor(
                                vsb[:, sb, :, 0:DH],
                                ps[:].rearrange("p (h d) -> p h d", h=HPC),
                                bvb_t[:].rearrange("p (h d) -> p h d",
                                                   h=HPC),
                                op=OP.add)
                            pre_step(2 if n == 3 else 1)

                ctxA.__exit__(None, None, None)
                # ------------- Phase B + fused C, per q-chunk -------------
                with (
                    tc.tile_pool(name="wrk", bufs=2) as wrk,
                    tc.tile_pool(name="psB", bufs=1,
                                 space="PSUM") as psB,
                ):
                    srowp = rcpp = toddp = obp = wrk
                    psAt = psO = psB
                    for ch in range(NCH):
                        ctxB = nc.named_scope(f"chunk{ch}")
                        ctxB.__enter__()
                        js = list(range(4 * ch, SBLK))
                        psa = [psAt.tile([DH + 1, 512], f32,
                                         tag=f"psa{h}", name=f"psa{h}")
                               for h in range(HPC)]
                        for idx, j in enumerate(js):
                            d = j - 4 * ch
                            W = 128 * (d + 1) if d < 4 else 512
                            slot = pre_map.get((ch, j))
                            if slot is not None:
                                pts = [ptbuf[:, slot, h, :]
                                       for h in range(HPC)]
                            else:
                                pts = [pers.tile([128, 512], bf16,
                                                 name="pt", tag="pt",
                                                 bufs=6)
                                       for _ in range(HPC)]
                                emit_score_block(ch, j, pts)
                            last = (idx == len(js) - 1) and ch < 3
                            for h in range(HPC):
                                nc.tensor.matmul(
                                    psa[h][:, 0:W], vsb[:, j, h, :],
                                    pts[h][:, 0:W],
                                    start=(idx == 0), stop=last,
                                    skip_group_check=(ch == 3))
                            for c_, j_, s_ in PRE_B.get((ch, idx), []):
                                emit_score_block(
                                    c_, j_,
                                    [ptbuf[:, s_, h_, :]
                                     for h_ in range(HPC)])
                        if ch == 3:
                            # last global row q=2047 is fully masked; its
                            # exact value (uniform attention = mean(V)@Wo)
                            # is recomputed on the host.  Keep column
                            # 511's denominator finite (one [0|1]-column
                            # matmul) to avoid Inf/NaN noise.
                            for h in range(HPC):
                                nc.tensor.matmul(
                                    psa[h][:, 510:512],
                                    vsb[:, 0, h, :], ones2[:],
                                    start=False, stop=True)
                        # normalize: attn^T rows / sums row.  Broadcast
                        # the sums row via a K=1 ones matmul, 64-lane
                        # approx reciprocal, then multiply.  Odd heads go
                        # through a SBUF tile and a partition-shifting DMA
                        # into rows 64:128 of the pair tile so phase C
                        # contracts K=128.  bcs tiles allocate from the
                        # psO pool (NOT psS) so the next chunk's first
                        # score matmul never waits on a late reciprocal
                        # read of a psS bank.
                        atn = pers.tile([128, NPAIR, 512], bf16,
                                        name="atn", tag="atn", bufs=2)
                        srows = []
                        for h in range(HPC):
                            srow = srowp.tile([DH + 1, 512], bf16)
                            nc.scalar.copy(srow[DH:DH + 1, :],
                                           psa[h][DH:DH + 1, :])
                            srows.append(srow)
                        rcps = []
                        for h in range(HPC):
                            bcs = psO.tile([128, 512], f32, name="bcs",
                                           tag="po", bufs=2)
                            # zero-add filler first: depends only on the
                            # psum bank, so the PE stays busy (and HAM
                            # stays hot) while the srow copy lands
                            zw = bcs[:, :] if ch == 3 else bcs[0:64, :]
                            zs = dmy_z[:, :] if ch == 3 else dmy_z[:, 0:64]
                            nc.tensor.matmul(zw, zs, dmy_m[:],
                                             start=True, stop=False)
                            nc.tensor.matmul(bcs[0:64, :],
                                             onesrow[DH:DH + 1, :],
                                             srows[h][DH:DH + 1, :],
                                             start=False, stop=True)
                            rcp = rcpp.tile([64, 512], f32)
                            nc.vector.reciprocal_approx_fast(rcp[:],
                                                             bcs[0:64, :])
                            rcps.append(rcp)
                        for h in range(HPC):
                            pair, half = h // 2, h % 2
                            if half == 0:
                                nc.vector.tensor_tensor(
                                    atn[0:64, pair, :], psa[h][0:DH, :],
                                    rcps[h][:], op=OP.mult)
                            else:
                                todd = toddp.tile([64, 512], bf16)
                                nc.vector.tensor_tensor(
                                    todd[:], psa[h][0:DH, :], rcps[h][:],
                                    op=OP.mult)
                                nc.sync.dma_start(atn[64:128, pair, :],
                                                  todd[:])

                        # fused phase C for this chunk's 4 s-blocks.  The
                        # first two psum groups are pre-opened with
                        # zero-add dummy matmuls: they depend only on free
                        # psO banks, so they fill the PE bubble while the
                        # norm chain (copy -> bcast -> rcp -> mult ->
                        # shift DMA) drains, and keep HAM warm.
                        for k in range(4):
                            sb = 4 * ch + k
                            ob = obp.tile([128, 2, 512], f32, name="ob",
                                          tag="ob", bufs=2)
                            for n in range(2):
                                ps = psO.tile([128, 512], f32,
                                              name="po", tag="po",
                                              bufs=2)
                                warm = (CWARM if k == 0 else
                                        (4 if ch == 3 else 0))
                                for w in range(warm):
                                    nc.tensor.matmul(
                                        ps[:], dmy_z[:], dmy_m[:],
                                        start=(w == 0), stop=False)
                                for p in range(NPAIR):
                                    nc.tensor.matmul(
                                        ps[:],
                                        atn[:, p, 128 * k:128 * k + 128],
                                        wo_t[:, p, 512 * n:512 * n + 512],
                                        start=(warm == 0 and p == 0),
                                        stop=(p == NPAIR - 1))
                                if n == 0:
                                    nc.scalar.copy(ob[:, 0, :], ps[:])
                                else:
                                    nc.vector.tensor_copy(ob[:, 1, :],
                                                          ps[:])
                            if ch == 3 and k == 3:
                                # final store: split across both rings so
                                # the drain isn\'t one serial 512KB DMA
                                nc.sync.dma_start(
                                    out[128 * sb:128 * sb + 128, 0:512],
                                    ob[:, 0, :])
                                nc.scalar.dma_start(
                                    out[128 * sb:128 * sb + 128,
                                        512:1024], ob[:, 1, :])
                            else:
                                nc.sync.dma_start(
                                    out[128 * sb:128 * sb + 128, :],
                                    ob[:].rearrange("p a b -> p (a b)"))
                        ctxB.__exit__(None, None, None)

    nc.finalize()
    return nc


def _prep_in_maps(inputs, Wq, bq, Wk, bk, Wv, bv, Wo, bo):
    import ml_dtypes
    bf = ml_dtypes.bfloat16
    in_maps = []
    # xq[n, p, c, s] = x^T[128c+p, 512n+s]: each (quarter, partition) is
    # a contiguous 8KB run, so the DMA gets full-size descriptors
    xqs = []
    for b in range(B):
        xT = np.ascontiguousarray(inputs[b].T).astype(bf)
        xqs.append(np.ascontiguousarray(
            xT.reshape(CHUNKS, 128, NCH, 512).transpose(2, 1, 0, 3)))
    kk = np.arange(128)[:, None, None]
    dd = np.arange(4)[None, :, None]
    ff = np.arange(512)[None, None, :]
    masks = ((128 * dd + kk) > ff).astype(bf)
    for core in range(NCORES):
        b = core // (NCORES // B)
        g = core % (NCORES // B)
        cols = slice(g * HPC * DH, (g + 1) * HPC * DH)
        bq_c = bq[cols].reshape(NPAIR, 128).T          # [128, 2]
        bk_c = bk[cols].reshape(NPAIR, 128).T
        bqk_c = np.ascontiguousarray(
            np.concatenate([bq_c, bk_c], axis=1), dtype=np.float32)
        bvb_c = np.ascontiguousarray(
            np.broadcast_to(bv[cols][None, :], (128, HPC * DH)),
            dtype=np.float32)

        def pack_w(w):  # [D, HPC*DH] -> [128, CHUNKS, HPC*DH]
            return np.ascontiguousarray(
                w.astype(bf).reshape(CHUNKS, 128, HPC * DH)
                .transpose(1, 0, 2))

        wo_c = np.ascontiguousarray(
            Wo[cols, :].astype(bf).reshape(NPAIR, 128, D)
            .transpose(1, 0, 2))
        in_maps.append({
            "xq": xqs[b],
            "wq": pack_w(np.ascontiguousarray(Wq[:, cols])),
            "wk": pack_w(np.ascontiguousarray(Wk[:, cols])),
            "wv": pack_w(np.ascontiguousarray(Wv[:, cols])),
            "wo": wo_c,
            "bqk": bqk_c,
            "bvb": bvb_c,
            "masks": masks,
        })
    return in_maps


def kernel(inputs, Wq, bq, Wk, bk, Wv, bv, Wo, bo, _want_results=False,
           **_run_kwargs):
    from concourse.bass_utils import run_bass_kernel_spmd

    inputs = np.asarray(inputs, dtype=np.float32)
    Wq, bq = np.asarray(Wq, np.float32), np.asarray(bq, np.float32)
    Wk, bk = np.asarray(Wk, np.float32), np.asarray(bk, np.float32)
    Wv, bv = np.asarray(Wv, np.float32), np.asarray(bv, np.float32)
    Wo, bo = np.asarray(Wo, np.float32), np.asarray(bo, np.float32)

    if "nc" not in _CACHE:
        _CACHE["nc"] = _build_nc()
    nc = _CACHE["nc"]

    in_maps = _prep_in_maps(inputs, Wq, bq, Wk, bk, Wv, bv, Wo, bo)
    res = run_bass_kernel_spmd(nc, in_maps, core_ids=list(range(NCORES)),
                               **_run_kwargs)

    out = np.zeros((B, S, D), dtype=np.float32)
    for core in range(NCORES):
        b = core // (NCORES // B)
        out[b] += res.results[core]["out"]
    out += bo[None, None, :]
    # exact last row (fully masked -> uniform attention = mean(V) @ Wo)
    for b in range(B):
        v_mean = inputs[b].mean(axis=0) @ Wv + bv
        out[b, S - 1, :] = v_mean @ Wo + bo
    if _want_results:
        return out, res
    return out

